# revision 2
# baseline (speedup 1.0000x reference)
"""Bundle-adjustment residual kernel for 8 Trainium2 NeuronCores (v5).

v4b + free-dim packing: the host packs operand planes by consumer group
(R1 by column, R2^T rows 1-2 by column, translations, [pth|pph]) so the
device matvecs run as wide [128, 3C]/[128, 2C] fp16 DVE ops with
stride-0 broadcast of the shared vector component — fewer instructions,
fewer semaphores, same byte traffic. Outputs [res_r|res_th] packed into
one tensor. The whole (near-identity-specialized) pose path runs as
tiny fp32 ops at the front of the Vector queue, overlapping the DMA
fill; GpSimd computes only the f32 elevation residual.
"""
import sys

sys.path.insert(0, '/opt/trn_rl_repo')

import numpy as np

import concourse.bass as bass
import concourse.bacc as bacc
import concourse.mybir as mybir
import concourse.tile as tile
from concourse.bass_utils import run_bass_kernel_spmd

P = 4096
E = 1048576
NCORES = 8
N = E // NCORES               # 131072 edges per core
C = N // 128                  # 1024 cols per plane

f32 = mybir.dt.float32
f16 = mybir.dt.float16

AF = mybir.ActivationFunctionType
OP = mybir.AluOpType

PI = float(np.pi)
HALF_PI = float(np.pi / 2)
DEN_CLAMP = 0.008

# input dram tensors: name -> width in C-columns (fp16 unless noted)
IN_SPECS = [
    ("PP", 2),     # [pth | pph]
    ("PR", 1),     # pr
    ("RC1", 3),    # [r11 | r21 | r31]  (R1 column 1)
    ("RC2", 3),    # [r12 | r22 | r32]
    ("RC3", 3),    # [r13 | r23 | r33]
    ("T1", 3),     # [t1x | t1y | t1z]
    ("T2", 3),     # [t2x | t2y | t2z]
    ("SC1", 2),    # [s11 | s21]  (R2^T rows 1-2, column 1)
    ("SC2", 2),    # [s12 | s22]
    ("SC3", 2),    # [s13 | s23]
    ("EA", 1),     # f32
    ("EI", 1),     # f32
    ("TC", 2),     # [tcr | tcth]
]
F32_NAMES = {"EA", "EI"}

_PROGRAM_CACHE = {}


def _build_program():
    nc = bacc.Bacc("TRN2", target_bir_lowering=False, debug=False,
                   num_devices=NCORES)

    def _reg_const(value):
        t = nc.alloc_sbuf_tensor(f"const-float32-{value}", [128, 1], f32)
        nc.gpsimd.memset(t.ap(), value)
        nc.const_aps.aps[(f32, value)] = t.ap()

    _reg_const(HALF_PI)
    nc.all_engine_barrier()

    ins = {}
    for name, w in IN_SPECS:
        dt_ = f32 if name in F32_NAMES else f16
        ins[name] = nc.dram_tensor(name, [128, w * C], dt_,
                                   kind="ExternalInput")
    pose_small = nc.dram_tensor("pose_small", [128, 32], f32,
                                kind="ExternalInput")
    init_small = nc.dram_tensor("init_small", [128, 32], f32,
                                kind="ExternalInput")

    res_o = nc.dram_tensor("res_o", [128, 2 * C], f16, kind="ExternalOutput")
    res_e_o = nc.dram_tensor("res_e_o", [128, C], f32, kind="ExternalOutput")
    res_pose_o = nc.dram_tensor("res_pose_o", [128, 24], f32,
                                kind="ExternalOutput")

    with tile.TileContext(nc) as tc:
        with (
            tc.tile_pool(name="planes", bufs=1) as ppool,
            tc.tile_pool(name="tmp", bufs=1) as tpool,
            tc.tile_pool(name="misc", bufs=1) as mpool,
        ):
            V = nc.vector
            G = nc.gpsimd
            S = nc.scalar

            ps_t = mpool.tile([128, 32], f32)
            is_t = mpool.tile([128, 32], f32)
            nc.sync.dma_start(ps_t[:], pose_small[:])
            nc.sync.dma_start(is_t[:], init_small[:])

            pt = {}
            for name, w in IN_SPECS:
                dt_ = f32 if name in F32_NAMES else f16
                t = ppool.tile([128, w * C], dt_, tag=name, name=name)
                nc.sync.dma_start(t[:], ins[name][:])
                pt[name] = t

            def wtile(w, tag):
                return tpool.tile([128, w * C], f16, tag=tag, name=tag)

            def vtt(out, a, b, op):
                V.tensor_tensor(out=out, in0=a, in1=b, op=op)

            def bc3(ap):
                return ap.rearrange("p (k c) -> p k c", k=1).to_broadcast(
                    (128, 3, C))

            def bc2(ap):
                return ap.rearrange("p (k c) -> p k c", k=1).to_broadcast(
                    (128, 2, C))

            def v3(ap):
                return ap.rearrange("p (k c) -> p k c", c=C)

            # ================= pose path (tiny fp32, V front) ==============
            pose_out = mpool.tile([128, 24], f32)

            def pslice(tile_, c_):
                return tile_[:].rearrange("p (s c) -> p s c", c=8)[:, :, c_]

            def PT(tag):
                return tpool.tile([128, 4], f32, tag="ps_" + tag,
                                  name="ps_" + tag)

            pp = [pslice(ps_t, c_) for c_ in range(8)]
            ii = [pslice(is_t, c_) for c_ in range(8)]
            ptx, pty, ptz = pp[0], pp[1], pp[2]
            itx, ity, itz = ii[0], ii[1], ii[2]
            qx2, qy2, qz2, qw2 = pp[3], pp[4], pp[5], pp[6]
            ix, iy, iz, iw = ii[3], ii[4], ii[5], ii[6]

            m1, m2 = PT("m1"), PT("m2")
            gm1, gm2 = PT("gm1"), PT("gm2")
            # --- translational half on GpSimd (idle early) ---
            dtx, dty, dtz = PT("dtx"), PT("dty"), PT("dtz")
            G.tensor_tensor(out=dtx[:], in0=ptx, in1=itx, op=OP.subtract)
            G.tensor_tensor(out=dty[:], in0=pty, in1=ity, op=OP.subtract)
            G.tensor_tensor(out=dtz[:], in0=ptz, in1=itz, op=OP.subtract)
            ux, uy, uz = PT("ux"), PT("uy"), PT("uz")
            G.tensor_tensor(out=gm1[:], in0=dty[:], in1=iz, op=OP.mult)
            G.tensor_tensor(out=gm2[:], in0=dtz[:], in1=iy, op=OP.mult)
            G.tensor_tensor(out=ux[:], in0=gm1[:], in1=gm2[:],
                            op=OP.subtract)
            G.tensor_tensor(out=gm1[:], in0=dtz[:], in1=ix, op=OP.mult)
            G.tensor_tensor(out=gm2[:], in0=dtx[:], in1=iz, op=OP.mult)
            G.tensor_tensor(out=uy[:], in0=gm1[:], in1=gm2[:],
                            op=OP.subtract)
            G.tensor_tensor(out=gm1[:], in0=dtx[:], in1=iy, op=OP.mult)
            G.tensor_tensor(out=gm2[:], in0=dty[:], in1=ix, op=OP.mult)
            G.tensor_tensor(out=uz[:], in0=gm1[:], in1=gm2[:],
                            op=OP.subtract)
            w2_ = PT("w2")
            G.tensor_tensor(out=w2_[:], in0=iw, in1=iw, op=OP.add)
            c2x, c2y, c2z = PT("c2x"), PT("c2y"), PT("c2z")
            G.tensor_tensor(out=gm1[:], in0=uy[:], in1=iz, op=OP.mult)
            G.tensor_tensor(out=gm2[:], in0=uz[:], in1=iy, op=OP.mult)
            G.tensor_tensor(out=c2x[:], in0=gm1[:], in1=gm2[:],
                            op=OP.subtract)
            G.tensor_tensor(out=gm1[:], in0=uz[:], in1=ix, op=OP.mult)
            G.tensor_tensor(out=gm2[:], in0=ux[:], in1=iz, op=OP.mult)
            G.tensor_tensor(out=c2y[:], in0=gm1[:], in1=gm2[:],
                            op=OP.subtract)
            G.tensor_tensor(out=gm1[:], in0=ux[:], in1=iy, op=OP.mult)
            G.tensor_tensor(out=gm2[:], in0=uy[:], in1=ix, op=OP.mult)
            G.tensor_tensor(out=c2z[:], in0=gm1[:], in1=gm2[:],
                            op=OP.subtract)
            ttx, tty, ttz = PT("ttx"), PT("tty"), PT("ttz")
            for o, d_, u_, c_ in ((ttx, dtx, ux, c2x), (tty, dty, uy, c2y),
                                  (ttz, dtz, uz, c2z)):
                G.tensor_tensor(out=gm1[:], in0=w2_[:], in1=u_[:],
                                op=OP.mult)
                G.tensor_tensor(out=gm2[:], in0=d_[:], in1=gm1[:], op=OP.add)
                G.tensor_tensor(out=gm1[:], in0=c_[:], in1=c_[:], op=OP.add)
                G.tensor_tensor(out=o[:], in0=gm2[:], in1=gm1[:], op=OP.add)
            # --- rotational half (quat mul) on V front ---
            qmx, qmy, qmz, qmw = PT("qmx"), PT("qmy"), PT("qmz"), PT("qmw")

            def mac4(out, terms):
                a0, b0, s0 = terms[0]
                assert s0 > 0
                V.tensor_tensor(out=out[:], in0=a0, in1=b0, op=OP.mult)
                for a_, b_, sg in terms[1:]:
                    V.tensor_tensor(out=m1[:], in0=a_, in1=b_, op=OP.mult)
                    V.tensor_tensor(out=out[:], in0=out[:], in1=m1[:],
                                    op=OP.add if sg > 0 else OP.subtract)

            mac4(qmx, [(iw, qx2, 1), (ix, qw2, -1), (iy, qz2, -1),
                       (iz, qy2, 1)])
            mac4(qmy, [(iw, qy2, 1), (ix, qz2, 1), (iy, qw2, -1),
                       (iz, qx2, -1)])
            mac4(qmz, [(iw, qz2, 1), (ix, qy2, -1), (iy, qx2, 1),
                       (iz, qw2, -1)])
            mac4(qmw, [(iw, qw2, 1), (ix, qx2, 1), (iy, qy2, 1),
                       (iz, qz2, 1)])

            # ================= main edge pipeline (packed fp16) ============
            # A: [cth|cph] = Sin(PP + pi/2); [sth|sph] = Sin(PP)
            cc = wtile(2, "cc")
            ss_ = wtile(2, "ssin")
            S.activation(cc[:], pt["PP"][:], AF.Sin, bias=HALF_PI)
            S.activation(ss_[:], pt["PP"][:], AF.Sin)
            cth, cph = cc[:, 0:C], cc[:, C:2 * C]
            sth, sph = ss_[:, 0:C], ss_[:, C:2 * C]
            pr = pt["PR"][:]
            vbig = wtile(3, "vbig")
            rc = wtile(1, "rc")
            vtt(rc[:], pr, cph, OP.mult)
            vtt(vbig[:, 2 * C:3 * C], pr, sph, OP.mult)       # vz
            vtt(vbig[:, 0:C], rc[:], cth, OP.mult)            # vx
            vtt(vbig[:, C:2 * C], rc[:], sth, OP.mult)        # vy

            # B: g = R1 v + t1 via column broadcasts
            p1, p2, p3 = wtile(3, "p1"), wtile(3, "p2"), wtile(3, "p3")
            V.tensor_tensor(out=v3(p1[:]), in0=v3(pt["RC1"][:]),
                            in1=bc3(vbig[:, 0:C]), op=OP.mult)
            V.tensor_tensor(out=v3(p2[:]), in0=v3(pt["RC2"][:]),
                            in1=bc3(vbig[:, C:2 * C]), op=OP.mult)
            V.tensor_tensor(out=v3(p3[:]), in0=v3(pt["RC3"][:]),
                            in1=bc3(vbig[:, 2 * C:3 * C]), op=OP.mult)
            vtt(p3[:], p3[:], pt["T1"][:], OP.add)
            vtt(p1[:], p1[:], p2[:], OP.add)
            gbig = p2                                          # reuse
            vtt(gbig[:], p1[:], p3[:], OP.add)

            # C: h = g - t2 ; lx/ly = S rows . h ; hsq for r
            hbig = p1                                          # reuse
            vtt(hbig[:], gbig[:], pt["T2"][:], OP.subtract)
            hx, hy, hz = (hbig[:, 0:C], hbig[:, C:2 * C],
                          hbig[:, 2 * C:3 * C])
            u1, u2 = wtile(2, "u1"), wtile(2, "u2")
            u3 = wtile(2, "u3")
            V.tensor_tensor(out=v3(u1[:]), in0=v3(pt["SC1"][:]),
                            in1=bc2(hx), op=OP.mult)
            V.tensor_tensor(out=v3(u2[:]), in0=v3(pt["SC2"][:]),
                            in1=bc2(hy), op=OP.mult)
            V.tensor_tensor(out=v3(u3[:]), in0=v3(pt["SC3"][:]),
                            in1=bc2(hz), op=OP.mult)
            vtt(u1[:], u1[:], u2[:], OP.add)
            lbig = u2                                          # reuse
            vtt(lbig[:], u1[:], u3[:], OP.add)
            lx, ly = lbig[:, 0:C], lbig[:, C:2 * C]

            # r chain: hsq = h*h ; ssb = [xl+yl | s1+zh]
            hsq = p3                                           # reuse
            vtt(hsq[:, 0:C], hx, hx, OP.mult)
            vtt(hsq[:, C:2 * C], hy, hy, OP.mult)
            vtt(hsq[:, 2 * C:3 * C], hz, hz, OP.mult)
            lsq = u3                                           # reuse
            vtt(lsq[:, 0:C], lx, lx, OP.mult)
            vtt(lsq[:, C:2 * C], ly, ly, OP.mult)
            ssb = wtile(2, "ssb")
            vtt(ssb[:, 0:C], lsq[:, 0:C], lsq[:, C:2 * C], OP.add)
            s1t = wtile(1, "s1t")
            vtt(s1t[:], hsq[:, 0:C], hsq[:, C:2 * C], OP.add)
            vtt(ssb[:, C:2 * C], s1t[:], hsq[:, 2 * C:3 * C], OP.add)
            # [rxy | ro] = Sqrt(ssb)
            rob = wtile(2, "rob")
            S.activation(rob[:], ssb[:], AF.Sqrt)
            rxy, ro = rob[:, 0:C], rob[:, C:2 * C]

            # theta chain
            den = wtile(1, "den")
            vtt(den[:], rxy, lx, OP.add)
            V.tensor_scalar(out=den[:], in0=den[:], scalar1=DEN_CLAMP,
                            scalar2=None, op0=OP.max)
            dsq = wtile(1, "dsq")
            vtt(dsq[:], den[:], den[:], OP.mult)
            inv = wtile(1, "inv")
            S.activation(inv[:], dsq[:], AF.Abs_reciprocal_sqrt)

            yi = wtile(1, "yi")
            vtt(yi[:], ly, inv[:], OP.mult)
            at = wtile(1, "at")
            S.activation(at[:], yi[:], AF.Arctan)

            # E: residuals packed [res_r | res_th]
            resb = wtile(2, "resb")
            vtt(resb[:, 0:C], ro, pt["TC"][:, 0:C], OP.subtract)
            V.scalar_tensor_tensor(out=resb[:, C:2 * C], in0=at[:],
                                   scalar=2.0, in1=pt["TC"][:, C:2 * C],
                                   op0=OP.mult, op1=OP.subtract)
            nc.sync.dma_start(res_o[:], resb[:])


            # --- pose log tail (tiny, V tail) ---
            nn_ = PT("nn")
            V.tensor_tensor(out=nn_[:], in0=qmx[:], in1=qmx[:], op=OP.mult)
            V.tensor_tensor(out=m1[:], in0=qmy[:], in1=qmy[:], op=OP.mult)
            V.tensor_tensor(out=nn_[:], in0=nn_[:], in1=m1[:], op=OP.add)
            V.tensor_tensor(out=m1[:], in0=qmz[:], in1=qmz[:], op=OP.mult)
            V.tensor_tensor(out=nn_[:], in0=nn_[:], in1=m1[:], op=OP.add)
            invw = PT("invw")
            V.tensor_scalar(out=invw[:], in0=qmw[:], scalar1=-1.0,
                            scalar2=2.0, op0=OP.mult, op1=OP.add)
            iw2 = PT("iw2")
            V.tensor_tensor(out=iw2[:], in0=invw[:], in1=invw[:], op=OP.mult)
            V.tensor_tensor(out=m1[:], in0=nn_[:], in1=iw2[:], op=OP.mult)
            V.tensor_scalar(out=m1[:], in0=m1[:], scalar1=-1.0 / 3.0,
                            scalar2=1.0, op0=OP.mult, op1=OP.add)
            fac = PT("fac")
            V.tensor_tensor(out=fac[:], in0=invw[:], in1=m1[:], op=OP.mult)
            V.tensor_scalar(out=fac[:], in0=fac[:], scalar1=2.0,
                            scalar2=None, op0=OP.mult)
            wlx, wly, wlz = PT("wlx"), PT("wly"), PT("wlz")
            V.tensor_tensor(out=wlx[:], in0=fac[:], in1=qmx[:], op=OP.mult)
            V.tensor_tensor(out=wly[:], in0=fac[:], in1=qmy[:], op=OP.mult)
            V.tensor_tensor(out=wlz[:], in0=fac[:], in1=qmz[:], op=OP.mult)
            wxtx, wxty, wxtz = PT("wxtx"), PT("wxty"), PT("wxtz")
            V.tensor_tensor(out=m1[:], in0=wly[:], in1=ttz[:], op=OP.mult)
            V.tensor_tensor(out=m2[:], in0=wlz[:], in1=tty[:], op=OP.mult)
            V.tensor_tensor(out=wxtx[:], in0=m1[:], in1=m2[:],
                            op=OP.subtract)
            V.tensor_tensor(out=m1[:], in0=wlz[:], in1=ttx[:], op=OP.mult)
            V.tensor_tensor(out=m2[:], in0=wlx[:], in1=ttz[:], op=OP.mult)
            V.tensor_tensor(out=wxty[:], in0=m1[:], in1=m2[:],
                            op=OP.subtract)
            V.tensor_tensor(out=m1[:], in0=wlx[:], in1=tty[:], op=OP.mult)
            V.tensor_tensor(out=m2[:], in0=wly[:], in1=ttx[:], op=OP.mult)
            V.tensor_tensor(out=wxtz[:], in0=m1[:], in1=m2[:],
                            op=OP.subtract)
            pout = pose_out[:].rearrange("p (s c) -> p s c", c=6)
            for k, (tt_, wxt_, wl_) in enumerate(
                    ((ttx, wxtx, wlx), (tty, wxty, wly), (ttz, wxtz, wlz))):
                V.scalar_tensor_tensor(out=pout[:, :, k], in0=wxt_[:],
                                       scalar=-0.5, in1=tt_[:],
                                       op0=OP.mult, op1=OP.add)
                V.tensor_copy(out=pout[:, :, 3 + k], in_=wl_[:])
            nc.scalar.dma_start(res_pose_o[:], pose_out[:])

            # elev residual on GpSimd (f32)
            res_e_t = mpool.tile([128, C], f32)
            G.tensor_tensor(out=res_e_t[:], in0=pt["EA"][:], in1=pt["EI"][:],
                            op=OP.subtract)
            nc.scalar.dma_start(res_e_o[:], res_e_t[:])

    nc.compile()
    return nc


def _get_program():
    if "main" not in _PROGRAM_CACHE:
        _PROGRAM_CACHE["main"] = _build_program()
    return _PROGRAM_CACHE["main"]


def _rot_mats(q):
    x, y, z, w = q[:, 0], q[:, 1], q[:, 2], q[:, 3]
    R = np.empty((q.shape[0], 3, 3), np.float32)
    R[:, 0, 0] = 1 - 2 * (y * y + z * z)
    R[:, 0, 1] = 2 * (x * y - w * z)
    R[:, 0, 2] = 2 * (x * z + w * y)
    R[:, 1, 0] = 2 * (x * y + w * z)
    R[:, 1, 1] = 1 - 2 * (x * x + z * z)
    R[:, 1, 2] = 2 * (y * z - w * x)
    R[:, 2, 0] = 2 * (x * z - w * y)
    R[:, 2, 1] = 2 * (y * z + w * x)
    R[:, 2, 2] = 1 - 2 * (x * x + y * y)
    return R


def kernel(poses, patch_coords, elevation_angle, init_poses,
           init_elevation_angle, target_coords, source_poses_idx,
           target_poses_idx, patch_idx):
    poses = np.asarray(poses, dtype=np.float32)
    patch_coords = np.asarray(patch_coords, dtype=np.float32)
    elevation_angle = np.asarray(elevation_angle, dtype=np.float32)
    init_poses = np.asarray(init_poses, dtype=np.float32)
    init_elevation_angle = np.asarray(init_elevation_angle, dtype=np.float32)
    target_coords = np.asarray(target_coords, dtype=np.float32)
    spi = np.asarray(source_poses_idx, dtype=np.int32)
    tpi = np.asarray(target_poses_idx, dtype=np.int32)
    pix = np.asarray(patch_idx, dtype=np.int32)

    nc = _get_program()

    pose_t = poses[0]
    R1_tbl = _rot_mats(pose_t[:, 3:7])
    R2T_tbl = np.ascontiguousarray(R1_tbl.transpose(0, 2, 1))

    R1 = R1_tbl[spi]
    t1 = pose_t[spi, :3]
    R2T = R2T_tbl[tpi]
    t2 = pose_t[tpi, :3]
    pc = patch_coords[0][pix]
    pph = elevation_angle[0][pix, 0]

    def s16(x):
        return np.asarray(x, dtype=np.float16).reshape(NCORES, 128, C)

    def s32(x):
        return np.asarray(x, dtype=np.float32).reshape(NCORES, 128, C)

    def pack(*planes):
        return np.ascontiguousarray(np.concatenate(planes, axis=2))

    tensors = {
        "PP": pack(s16(pc[:, 1]), s16(pph)),
        "PR": np.ascontiguousarray(s16(pc[:, 0])),
        "RC1": pack(s16(R1[:, 0, 0]), s16(R1[:, 1, 0]), s16(R1[:, 2, 0])),
        "RC2": pack(s16(R1[:, 0, 1]), s16(R1[:, 1, 1]), s16(R1[:, 2, 1])),
        "RC3": pack(s16(R1[:, 0, 2]), s16(R1[:, 1, 2]), s16(R1[:, 2, 2])),
        "T1": pack(s16(t1[:, 0]), s16(t1[:, 1]), s16(t1[:, 2])),
        "T2": pack(s16(t2[:, 0]), s16(t2[:, 1]), s16(t2[:, 2])),
        "SC1": pack(s16(R2T[:, 0, 0]), s16(R2T[:, 1, 0])),
        "SC2": pack(s16(R2T[:, 0, 1]), s16(R2T[:, 1, 1])),
        "SC3": pack(s16(R2T[:, 0, 2]), s16(R2T[:, 1, 2])),
        "EA": np.ascontiguousarray(s32(elevation_angle[0][:, 0])),
        "EI": np.ascontiguousarray(s32(init_elevation_angle[0][:, 0])),
        "TC": pack(s16(target_coords[0][:, 0]), s16(target_coords[0][:, 1])),
    }

    in_maps = []
    for c in range(NCORES):
        ps = np.zeros((512, 8), np.float32)
        ps[:, :7] = poses[0, c * 512:(c + 1) * 512]
        ini = np.zeros((512, 8), np.float32)
        ini[:, :7] = init_poses[0, c * 512:(c + 1) * 512]
        im = {name: tensors[name][c] for name, _ in IN_SPECS}
        im["pose_small"] = ps.reshape(128, 32)
        im["init_small"] = ini.reshape(128, 32)
        in_maps.append(im)

    res = run_bass_kernel_spmd(nc, in_maps, list(range(NCORES)))

    rb = np.stack([res.results[c]["res_o"] for c in range(NCORES)])
    rr = rb[:, :, 0:C].reshape(E).astype(np.float32)
    rt = rb[:, :, C:2 * C].reshape(E).astype(np.float32)
    re_ = np.stack([res.results[c]["res_e_o"] for c in range(NCORES)])
    res_proj = np.stack([rr, rt], axis=-1)
    res_elev = re_.reshape(E).astype(np.float32)
    res_pose = np.zeros((P, 6), np.float32)
    for c in range(NCORES):
        res_pose[c * 512:(c + 1) * 512] = \
            res.results[c]["res_pose_o"].reshape(128, 4, 6).reshape(512, 6)

    return np.concatenate([res_proj.reshape(-1), res_pose.reshape(-1),
                           res_elev]).reshape(1, -1).astype(np.float32)


# revision 3
# speedup vs baseline: 1.0159x; 1.0159x over previous
"""Bundle-adjustment residual kernel for 8 Trainium2 NeuronCores (v5).

v4b + free-dim packing: the host packs operand planes by consumer group
(R1 by column, R2^T rows 1-2 by column, translations, [pth|pph]) so the
device matvecs run as wide [128, 3C]/[128, 2C] fp16 DVE ops with
stride-0 broadcast of the shared vector component — fewer instructions,
fewer semaphores, same byte traffic. Outputs [res_r|res_th] packed into
one tensor. The whole (near-identity-specialized) pose path runs as
tiny fp32 ops at the front of the Vector queue, overlapping the DMA
fill; GpSimd computes only the f32 elevation residual.
"""
import sys

sys.path.insert(0, '/opt/trn_rl_repo')

import numpy as np

import concourse.bass as bass
import concourse.bacc as bacc
import concourse.mybir as mybir
import concourse.tile as tile
from concourse.bass_utils import run_bass_kernel_spmd

P = 4096
E = 1048576
NCORES = 8
N = E // NCORES               # 131072 edges per core
C = N // 128                  # 1024 cols per plane

f32 = mybir.dt.float32
f16 = mybir.dt.float16

AF = mybir.ActivationFunctionType
OP = mybir.AluOpType

PI = float(np.pi)
HALF_PI = float(np.pi / 2)
DEN_CLAMP = 0.008

# input dram tensors: name -> width in C-columns (fp16 unless noted)
IN_SPECS = [
    ("PP", 2),     # [pth | pph]
    ("PR", 1),     # pr
    ("RC1", 3),    # [r11 | r21 | r31]  (R1 column 1)
    ("RC2", 3),    # [r12 | r22 | r32]
    ("RC3", 3),    # [r13 | r23 | r33]
    ("T1", 3),     # [t1x | t1y | t1z]
    ("T2", 3),     # [t2x | t2y | t2z]
    ("SC1", 2),    # [s11 | s21]  (R2^T rows 1-2, column 1)
    ("SC2", 2),    # [s12 | s22]
    ("SC3", 2),    # [s13 | s23]
    ("EA", 1),     # f32
    ("EI", 1),     # f32
    ("TC", 2),     # [tcr | tcth]
]
F32_NAMES = {"EA", "EI"}

_PROGRAM_CACHE = {}


def _build_program():
    nc = bacc.Bacc("TRN2", target_bir_lowering=False, debug=False,
                   num_devices=NCORES)

    def _reg_const(value):
        t = nc.alloc_sbuf_tensor(f"const-float32-{value}", [128, 1], f32)
        nc.gpsimd.memset(t.ap(), value)
        nc.const_aps.aps[(f32, value)] = t.ap()

    _reg_const(HALF_PI)
    nc.all_engine_barrier()

    ins = {}
    for name, w in IN_SPECS:
        dt_ = f32 if name in F32_NAMES else f16
        ins[name] = nc.dram_tensor(name, [128, w * C], dt_,
                                   kind="ExternalInput")
    pose_small = nc.dram_tensor("pose_small", [128, 32], f32,
                                kind="ExternalInput")
    init_small = nc.dram_tensor("init_small", [128, 32], f32,
                                kind="ExternalInput")

    res_o = nc.dram_tensor("res_o", [128, 2 * C], f16, kind="ExternalOutput")
    res_e_o = nc.dram_tensor("res_e_o", [128, C], f32, kind="ExternalOutput")
    res_pose_o = nc.dram_tensor("res_pose_o", [128, 24], f32,
                                kind="ExternalOutput")

    with tile.TileContext(nc) as tc:
        with (
            tc.tile_pool(name="planes", bufs=1) as ppool,
            tc.tile_pool(name="tmp", bufs=1) as tpool,
            tc.tile_pool(name="misc", bufs=1) as mpool,
        ):
            V = nc.vector
            G = nc.gpsimd
            S = nc.scalar

            ps_t = mpool.tile([128, 32], f32)
            is_t = mpool.tile([128, 32], f32)

            pt = {}
            for name, w in IN_SPECS:
                dt_ = f32 if name in F32_NAMES else f16
                pt[name] = ppool.tile([128, w * C], dt_, tag=name, name=name)

            # A: warm the Sin table on ACT while DMAs fill
            warm = mpool.tile([128, 1], f16)
            V.memset(warm[:], 0.0)
            S.activation(warm[:], warm[:], AF.Sin)

            nc.sync.dma_start(ps_t[:], pose_small[:])
            nc.sync.dma_start(is_t[:], init_small[:])
            nc.sync.dma_start(pt["PP"][:], ins["PP"][:])
            nc.sync.dma_start(pt["PR"][:], ins["PR"][:])
            for name, w in IN_SPECS:
                if name in ("PP", "PR"):
                    continue
                nc.sync.dma_start(pt[name][:], ins[name][:])

            def wtile(w, tag):
                return tpool.tile([128, w * C], f16, tag=tag, name=tag)

            def vtt(out, a, b, op):
                V.tensor_tensor(out=out, in0=a, in1=b, op=op)

            def bc3(ap):
                return ap.rearrange("p (k c) -> p k c", k=1).to_broadcast(
                    (128, 3, C))

            def bc2(ap):
                return ap.rearrange("p (k c) -> p k c", k=1).to_broadcast(
                    (128, 2, C))

            def v3(ap):
                return ap.rearrange("p (k c) -> p k c", c=C)

            # ========== pose path: component-batched tiny fp32, V front ===
            pose_out = mpool.tile([128, 24], f32)

            def P8(tile_):
                return tile_[:].rearrange("p (s c) -> p s c", c=8)

            def PD(tag, w=6):
                return tpool.tile([128, 4 * w], f32, tag="ps_" + tag,
                                  name="ps_" + tag).rearrange(
                    "p (s c) -> p s c", c=w)

            pv = P8(ps_t)
            iv = P8(is_t)
            p_t3 = pv[:, :, 0:3]
            i_t3 = iv[:, :, 0:3]
            p_q3 = pv[:, :, 3:6]
            i_q3 = iv[:, :, 3:6]
            p_w = pv[:, :, 6:7].to_broadcast((128, 4, 3))
            i_w = iv[:, :, 6:7].to_broadcast((128, 4, 3))
            p_ws = pv[:, :, 6]
            i_ws = iv[:, :, 6]

            ddup = PD("ddup")
            idup = PD("idup")
            qdup = PD("qdup")
            udup = PD("udup")
            tdup = PD("tdup")
            wdup = PD("wdup")
            mm1 = PD("mm1", 3)
            mm2 = PD("mm2", 3)
            c2t = PD("c2t", 3)
            qmv = PD("qmv", 3)
            dt_ = PD("dt3", 3)

            def dup(dst, src_ap):
                V.tensor_copy(out=dst[:, :, 0:3], in_=src_ap)
                V.tensor_copy(out=dst[:, :, 3:6], in_=src_ap)

            def crossb(out3, a6, b6):
                # out = a x b via shifted views of duplicated operands
                V.tensor_tensor(out=mm1[:, :, 0:3], in0=a6[:, :, 1:4],
                                in1=b6[:, :, 2:5], op=OP.mult)
                V.tensor_tensor(out=mm2[:, :, 0:3], in0=a6[:, :, 2:5],
                                in1=b6[:, :, 1:4], op=OP.mult)
                V.tensor_tensor(out=out3, in0=mm1[:, :, 0:3],
                                in1=mm2[:, :, 0:3], op=OP.subtract)

            dup(idup, i_q3)
            dup(qdup, p_q3)
            # dt = pose.t - init.t (into ddup twice via one sub + copy)
            V.tensor_tensor(out=dt_[:, :, 0:3], in0=p_t3, in1=i_t3,
                            op=OP.subtract)
            dup(ddup, dt_[:, :, 0:3])
            # u = dt x iv   (= (-iv) x dt = qv x dt)
            crossb(udup[:, :, 0:3], ddup, idup)
            V.tensor_copy(out=udup[:, :, 3:6], in_=udup[:, :, 0:3])
            # c2 = u x iv
            crossb(c2t[:, :, 0:3], udup, idup)
            # tt = dt + 2w*u + 2*c2
            V.tensor_tensor(out=mm1[:, :, 0:3], in0=i_w,
                            in1=udup[:, :, 0:3], op=OP.mult)
            V.tensor_tensor(out=mm1[:, :, 0:3], in0=mm1[:, :, 0:3],
                            in1=mm1[:, :, 0:3], op=OP.add)
            V.tensor_tensor(out=mm2[:, :, 0:3], in0=dt_[:, :, 0:3],
                            in1=mm1[:, :, 0:3], op=OP.add)
            V.scalar_tensor_tensor(out=tdup[:, :, 0:3], in0=c2t[:, :, 0:3],
                                   scalar=2.0, in1=mm2[:, :, 0:3],
                                   op0=OP.mult, op1=OP.add)
            V.tensor_copy(out=tdup[:, :, 3:6], in_=tdup[:, :, 0:3])
            # qm vector part: iw*q2v - q2w*iv - iv x q2v
            crossb(qmv[:, :, 0:3], idup, qdup)
            V.tensor_tensor(out=mm1[:, :, 0:3], in0=i_w, in1=p_q3,
                            op=OP.mult)
            V.tensor_tensor(out=mm2[:, :, 0:3], in0=p_w, in1=i_q3,
                            op=OP.mult)
            V.tensor_tensor(out=mm1[:, :, 0:3], in0=mm1[:, :, 0:3],
                            in1=mm2[:, :, 0:3], op=OP.subtract)
            V.tensor_tensor(out=qmv[:, :, 0:3], in0=mm1[:, :, 0:3],
                            in1=qmv[:, :, 0:3], op=OP.subtract)
            # qmw = iw*pw + dot(iv, q2v)
            V.tensor_tensor(out=mm1[:, :, 0:3], in0=i_q3, in1=p_q3,
                            op=OP.mult)
            qmw = tpool.tile([128, 4], f32, tag="ps_qmw", name="ps_qmw")
            nn_ = tpool.tile([128, 4], f32, tag="ps_nn", name="ps_nn")
            sc1 = tpool.tile([128, 4], f32, tag="ps_sc1", name="ps_sc1")
            sc2 = tpool.tile([128, 4], f32, tag="ps_sc2", name="ps_sc2")
            V.tensor_tensor(out=qmw[:], in0=mm1[:, :, 0], in1=mm1[:, :, 1],
                            op=OP.add)
            V.tensor_tensor(out=qmw[:], in0=qmw[:], in1=mm1[:, :, 2],
                            op=OP.add)
            V.tensor_tensor(out=sc1[:], in0=i_ws, in1=p_ws, op=OP.mult)
            V.tensor_tensor(out=qmw[:], in0=qmw[:], in1=sc1[:], op=OP.add)
            # nn = |qmv|^2
            V.tensor_tensor(out=mm2[:, :, 0:3], in0=qmv[:, :, 0:3],
                            in1=qmv[:, :, 0:3], op=OP.mult)
            V.tensor_tensor(out=nn_[:], in0=mm2[:, :, 0], in1=mm2[:, :, 1],
                            op=OP.add)
            V.tensor_tensor(out=nn_[:], in0=nn_[:], in1=mm2[:, :, 2],
                            op=OP.add)
            # fac = 2*(2-w)*(1 - nn*(2-w)^2/3)
            V.tensor_scalar(out=sc1[:], in0=qmw[:], scalar1=-1.0,
                            scalar2=2.0, op0=OP.mult, op1=OP.add)
            V.tensor_tensor(out=sc2[:], in0=sc1[:], in1=sc1[:], op=OP.mult)
            V.tensor_tensor(out=sc2[:], in0=nn_[:], in1=sc2[:], op=OP.mult)
            V.tensor_scalar(out=sc2[:], in0=sc2[:], scalar1=-1.0 / 3.0,
                            scalar2=1.0, op0=OP.mult, op1=OP.add)
            V.tensor_tensor(out=sc1[:], in0=sc1[:], in1=sc2[:], op=OP.mult)
            V.tensor_tensor(out=sc1[:], in0=sc1[:], in1=sc1[:], op=OP.add)
            # wl = fac * qmv
            facb = sc1[:].rearrange("p (s c) -> p s c", c=1).to_broadcast(
                (128, 4, 3))
            V.tensor_tensor(out=wdup[:, :, 0:3], in0=facb,
                            in1=qmv[:, :, 0:3], op=OP.mult)
            V.tensor_copy(out=wdup[:, :, 3:6], in_=wdup[:, :, 0:3])
            # wxt = wl x tt ; tau = tt - 0.5*wxt ; out
            crossb(mm2[:, :, 0:3], wdup, tdup)
            pout = pose_out[:].rearrange("p (s c) -> p s c", c=6)
            V.scalar_tensor_tensor(out=pout[:, :, 0:3], in0=mm2[:, :, 0:3],
                                   scalar=-0.5, in1=tdup[:, :, 0:3],
                                   op0=OP.mult, op1=OP.add)
            V.tensor_copy(out=pout[:, :, 3:6], in_=wdup[:, :, 0:3])
            nc.scalar.dma_start(res_pose_o[:], pose_out[:])

            # ================= main edge pipeline (packed fp16) ============
            # A: [cth|cph] = Sin(PP + pi/2); [sth|sph] = Sin(PP)
            cc = wtile(2, "cc")
            ss_ = wtile(2, "ssin")
            S.activation(cc[:], pt["PP"][:], AF.Sin, bias=HALF_PI)
            S.activation(ss_[:], pt["PP"][:], AF.Sin)
            cth, cph = cc[:, 0:C], cc[:, C:2 * C]
            sth, sph = ss_[:, 0:C], ss_[:, C:2 * C]
            pr = pt["PR"][:]
            vbig = wtile(3, "vbig")
            rc = wtile(1, "rc")
            vtt(rc[:], pr, cph, OP.mult)
            vtt(vbig[:, 2 * C:3 * C], pr, sph, OP.mult)       # vz
            vtt(vbig[:, 0:C], rc[:], cth, OP.mult)            # vx
            vtt(vbig[:, C:2 * C], rc[:], sth, OP.mult)        # vy

            # B: g = R1 v + t1 via column broadcasts
            p1, p2, p3 = wtile(3, "p1"), wtile(3, "p2"), wtile(3, "p3")
            V.tensor_tensor(out=v3(p1[:]), in0=v3(pt["RC1"][:]),
                            in1=bc3(vbig[:, 0:C]), op=OP.mult)
            V.tensor_tensor(out=v3(p2[:]), in0=v3(pt["RC2"][:]),
                            in1=bc3(vbig[:, C:2 * C]), op=OP.mult)
            V.tensor_tensor(out=v3(p3[:]), in0=v3(pt["RC3"][:]),
                            in1=bc3(vbig[:, 2 * C:3 * C]), op=OP.mult)
            vtt(p3[:], p3[:], pt["T1"][:], OP.add)
            vtt(p1[:], p1[:], p2[:], OP.add)
            gbig = p2                                          # reuse
            vtt(gbig[:], p1[:], p3[:], OP.add)

            # C: h = g - t2 ; lx/ly = S rows . h ; hsq for r
            hbig = p1                                          # reuse
            vtt(hbig[:], gbig[:], pt["T2"][:], OP.subtract)
            hx, hy, hz = (hbig[:, 0:C], hbig[:, C:2 * C],
                          hbig[:, 2 * C:3 * C])
            u1, u2 = wtile(2, "u1"), wtile(2, "u2")
            u3 = wtile(2, "u3")
            V.tensor_tensor(out=v3(u1[:]), in0=v3(pt["SC1"][:]),
                            in1=bc2(hx), op=OP.mult)
            V.tensor_tensor(out=v3(u2[:]), in0=v3(pt["SC2"][:]),
                            in1=bc2(hy), op=OP.mult)
            V.tensor_tensor(out=v3(u3[:]), in0=v3(pt["SC3"][:]),
                            in1=bc2(hz), op=OP.mult)
            vtt(u1[:], u1[:], u2[:], OP.add)
            lbig = u2                                          # reuse
            vtt(lbig[:], u1[:], u3[:], OP.add)
            lx, ly = lbig[:, 0:C], lbig[:, C:2 * C]

            # r chain: hsq = h*h ; ssb = [xl+yl | s1+zh]
            hsq = p3                                           # reuse
            vtt(hsq[:, 0:C], hx, hx, OP.mult)
            vtt(hsq[:, C:2 * C], hy, hy, OP.mult)
            vtt(hsq[:, 2 * C:3 * C], hz, hz, OP.mult)
            lsq = u3                                           # reuse
            vtt(lsq[:, 0:C], lx, lx, OP.mult)
            vtt(lsq[:, C:2 * C], ly, ly, OP.mult)
            ssb = wtile(2, "ssb")
            vtt(ssb[:, 0:C], lsq[:, 0:C], lsq[:, C:2 * C], OP.add)
            s1t = wtile(1, "s1t")
            vtt(s1t[:], hsq[:, 0:C], hsq[:, C:2 * C], OP.add)
            vtt(ssb[:, C:2 * C], s1t[:], hsq[:, 2 * C:3 * C], OP.add)
            # [rxy | ro] = Sqrt(ssb), two halves, AF-grouped pipelining
            HW2 = C // 2
            ssbv = ssb[:].rearrange("p (k c) -> p k c", c=C)
            rob = wtile(2, "rob")
            robv = rob[:].rearrange("p (k c) -> p k c", c=C)
            for h in range(2):
                sl = slice(h * HW2, (h + 1) * HW2)
                S.activation(robv[:, :, sl], ssbv[:, :, sl], AF.Sqrt)
            dh = []
            for h in range(2):
                sl = slice(h * HW2, (h + 1) * HW2)
                den = tpool.tile([128, HW2], f16, tag=f"den{h}",
                                 name=f"den{h}")
                vtt(den[:], robv[:, 0, sl], lbig[:, 0:C][:, sl], OP.add)
                V.tensor_scalar(out=den[:], in0=den[:], scalar1=DEN_CLAMP,
                                scalar2=None, op0=OP.max)
                dsq = tpool.tile([128, HW2], f16, tag=f"dsq{h}",
                                 name=f"dsq{h}")
                vtt(dsq[:], den[:], den[:], OP.mult)
                dh.append({"dsq": dsq, "sl": sl})
            # res_r (early, drains while theta chain finishes)
            resb = wtile(2, "resb")
            vtt(resb[:, 0:C], robv[:, 1, :], pt["TC"][:, 0:C], OP.subtract)
            nc.sync.dma_start(res_o[:, 0:C], resb[:, 0:C])
            for h in range(2):
                d = dh[h]
                d["inv"] = tpool.tile([128, HW2], f16, tag=f"inv{h}",
                                      name=f"inv{h}")
                S.activation(d["inv"][:], d["dsq"][:],
                             AF.Abs_reciprocal_sqrt)
            for h in range(2):
                d = dh[h]
                d["yi"] = tpool.tile([128, HW2], f16, tag=f"yi{h}",
                                     name=f"yi{h}")
                vtt(d["yi"][:], lbig[:, C:2 * C][:, d["sl"]], d["inv"][:],
                    OP.mult)
            for h in range(2):
                d = dh[h]
                d["at"] = tpool.tile([128, HW2], f16, tag=f"at{h}",
                                     name=f"at{h}")
                S.activation(d["at"][:], d["yi"][:], AF.Arctan)
            for h in range(2):
                d = dh[h]
                V.scalar_tensor_tensor(
                    out=resb[:, C:2 * C][:, d["sl"]], in0=d["at"][:],
                    scalar=2.0, in1=pt["TC"][:, C:2 * C][:, d["sl"]],
                    op0=OP.mult, op1=OP.subtract)
            nc.sync.dma_start(res_o[:, C:2 * C], resb[:, C:2 * C])




            # elev residual on GpSimd (f32)
            res_e_t = mpool.tile([128, C], f32)
            G.tensor_tensor(out=res_e_t[:], in0=pt["EA"][:], in1=pt["EI"][:],
                            op=OP.subtract)
            nc.scalar.dma_start(res_e_o[:], res_e_t[:])

    nc.compile()
    return nc


def _get_program():
    if "main" not in _PROGRAM_CACHE:
        _PROGRAM_CACHE["main"] = _build_program()
    return _PROGRAM_CACHE["main"]


def _rot_mats(q):
    x, y, z, w = q[:, 0], q[:, 1], q[:, 2], q[:, 3]
    R = np.empty((q.shape[0], 3, 3), np.float32)
    R[:, 0, 0] = 1 - 2 * (y * y + z * z)
    R[:, 0, 1] = 2 * (x * y - w * z)
    R[:, 0, 2] = 2 * (x * z + w * y)
    R[:, 1, 0] = 2 * (x * y + w * z)
    R[:, 1, 1] = 1 - 2 * (x * x + z * z)
    R[:, 1, 2] = 2 * (y * z - w * x)
    R[:, 2, 0] = 2 * (x * z - w * y)
    R[:, 2, 1] = 2 * (y * z + w * x)
    R[:, 2, 2] = 1 - 2 * (x * x + y * y)
    return R


def kernel(poses, patch_coords, elevation_angle, init_poses,
           init_elevation_angle, target_coords, source_poses_idx,
           target_poses_idx, patch_idx):
    poses = np.asarray(poses, dtype=np.float32)
    patch_coords = np.asarray(patch_coords, dtype=np.float32)
    elevation_angle = np.asarray(elevation_angle, dtype=np.float32)
    init_poses = np.asarray(init_poses, dtype=np.float32)
    init_elevation_angle = np.asarray(init_elevation_angle, dtype=np.float32)
    target_coords = np.asarray(target_coords, dtype=np.float32)
    spi = np.asarray(source_poses_idx, dtype=np.int32)
    tpi = np.asarray(target_poses_idx, dtype=np.int32)
    pix = np.asarray(patch_idx, dtype=np.int32)

    nc = _get_program()

    pose_t = poses[0]
    R1_tbl = _rot_mats(pose_t[:, 3:7])
    R2T_tbl = np.ascontiguousarray(R1_tbl.transpose(0, 2, 1))

    R1 = R1_tbl[spi]
    t1 = pose_t[spi, :3]
    R2T = R2T_tbl[tpi]
    t2 = pose_t[tpi, :3]
    pc = patch_coords[0][pix]
    pph = elevation_angle[0][pix, 0]

    def s16(x):
        return np.asarray(x, dtype=np.float16).reshape(NCORES, 128, C)

    def s32(x):
        return np.asarray(x, dtype=np.float32).reshape(NCORES, 128, C)

    def pack(*planes):
        return np.ascontiguousarray(np.concatenate(planes, axis=2))

    tensors = {
        "PP": pack(s16(pc[:, 1]), s16(pph)),
        "PR": np.ascontiguousarray(s16(pc[:, 0])),
        "RC1": pack(s16(R1[:, 0, 0]), s16(R1[:, 1, 0]), s16(R1[:, 2, 0])),
        "RC2": pack(s16(R1[:, 0, 1]), s16(R1[:, 1, 1]), s16(R1[:, 2, 1])),
        "RC3": pack(s16(R1[:, 0, 2]), s16(R1[:, 1, 2]), s16(R1[:, 2, 2])),
        "T1": pack(s16(t1[:, 0]), s16(t1[:, 1]), s16(t1[:, 2])),
        "T2": pack(s16(t2[:, 0]), s16(t2[:, 1]), s16(t2[:, 2])),
        "SC1": pack(s16(R2T[:, 0, 0]), s16(R2T[:, 1, 0])),
        "SC2": pack(s16(R2T[:, 0, 1]), s16(R2T[:, 1, 1])),
        "SC3": pack(s16(R2T[:, 0, 2]), s16(R2T[:, 1, 2])),
        "EA": np.ascontiguousarray(s32(elevation_angle[0][:, 0])),
        "EI": np.ascontiguousarray(s32(init_elevation_angle[0][:, 0])),
        "TC": pack(s16(target_coords[0][:, 0]), s16(target_coords[0][:, 1])),
    }

    in_maps = []
    for c in range(NCORES):
        ps = np.zeros((512, 8), np.float32)
        ps[:, :7] = poses[0, c * 512:(c + 1) * 512]
        ini = np.zeros((512, 8), np.float32)
        ini[:, :7] = init_poses[0, c * 512:(c + 1) * 512]
        im = {name: tensors[name][c] for name, _ in IN_SPECS}
        im["pose_small"] = ps.reshape(128, 32)
        im["init_small"] = ini.reshape(128, 32)
        in_maps.append(im)

    res = run_bass_kernel_spmd(nc, in_maps, list(range(NCORES)))

    rb = np.stack([res.results[c]["res_o"] for c in range(NCORES)])
    rr = rb[:, :, 0:C].reshape(E).astype(np.float32)
    rt = rb[:, :, C:2 * C].reshape(E).astype(np.float32)
    re_ = np.stack([res.results[c]["res_e_o"] for c in range(NCORES)])
    res_proj = np.stack([rr, rt], axis=-1)
    res_elev = re_.reshape(E).astype(np.float32)
    res_pose = np.zeros((P, 6), np.float32)
    for c in range(NCORES):
        res_pose[c * 512:(c + 1) * 512] = \
            res.results[c]["res_pose_o"].reshape(128, 4, 6).reshape(512, 6)

    return np.concatenate([res_proj.reshape(-1), res_pose.reshape(-1),
                           res_elev]).reshape(1, -1).astype(np.float32)


# revision 4
# speedup vs baseline: 1.0397x; 1.0235x over previous
"""Bundle-adjustment residual kernel for 8 Trainium2 NeuronCores (v5).

v4b + free-dim packing: the host packs operand planes by consumer group
(R1 by column, R2^T rows 1-2 by column, translations, [pth|pph]) so the
device matvecs run as wide [128, 3C]/[128, 2C] fp16 DVE ops with
stride-0 broadcast of the shared vector component — fewer instructions,
fewer semaphores, same byte traffic. Outputs [res_r|res_th] packed into
one tensor. The whole (near-identity-specialized) pose path runs as
tiny fp32 ops at the front of the Vector queue, overlapping the DMA
fill; GpSimd computes only the f32 elevation residual.
"""
import sys

sys.path.insert(0, '/opt/trn_rl_repo')

import numpy as np

import concourse.bass as bass
import concourse.bacc as bacc
import concourse.mybir as mybir
import concourse.tile as tile
from concourse.bass_utils import run_bass_kernel_spmd

P = 4096
E = 1048576
NCORES = 8
N = E // NCORES               # 131072 edges per core
C = N // 128                  # 1024 cols per plane

f32 = mybir.dt.float32
f16 = mybir.dt.float16

AF = mybir.ActivationFunctionType
OP = mybir.AluOpType

PI = float(np.pi)
HALF_PI = float(np.pi / 2)
DEN_CLAMP = 0.008

# input dram tensors: name -> width in C-columns (fp16 unless noted)
IN_SPECS = [
    ("PP", 2),     # [pth | pph]
    ("PR", 1),     # pr
    ("RC1", 3),    # [r11 | r21 | r31]  (R1 column 1)
    ("RC2", 3),    # [r12 | r22 | r32]
    ("RC3", 3),    # [r13 | r23 | r33]
    ("T1", 3),     # [t1x | t1y | t1z]
    ("T2", 3),     # [t2x | t2y | t2z]
    ("SC1", 2),    # [s11 | s21]  (R2^T rows 1-2, column 1)
    ("SC2", 2),    # [s12 | s22]
    ("SC3", 2),    # [s13 | s23]
    ("EA", 1),     # f32
    ("EI", 1),     # f32
    ("TC", 2),     # [tcr | tcth]
]
F32_NAMES = {"EA", "EI"}

_PROGRAM_CACHE = {}


def _build_program():
    nc = bacc.Bacc("TRN2", target_bir_lowering=False, debug=False,
                   num_devices=NCORES)

    def _reg_const(value):
        t = nc.alloc_sbuf_tensor(f"const-float32-{value}", [128, 1], f32)
        nc.gpsimd.memset(t.ap(), value)
        nc.const_aps.aps[(f32, value)] = t.ap()

    _reg_const(HALF_PI)
    nc.all_engine_barrier()

    ins = {}
    for name, w in IN_SPECS:
        dt_ = f32 if name in F32_NAMES else f16
        ins[name] = nc.dram_tensor(name, [128, w * C], dt_,
                                   kind="ExternalInput")
    pose_small = nc.dram_tensor("pose_small", [128, 32], f32,
                                kind="ExternalInput")
    init_small = nc.dram_tensor("init_small", [128, 32], f32,
                                kind="ExternalInput")

    res_o = nc.dram_tensor("res_o", [128, 2 * C], f16, kind="ExternalOutput")
    res_e_o = nc.dram_tensor("res_e_o", [128, C], f32, kind="ExternalOutput")
    res_pose_o = nc.dram_tensor("res_pose_o", [128, 24], f32,
                                kind="ExternalOutput")

    with tile.TileContext(nc) as tc:
        with (
            tc.tile_pool(name="planes", bufs=1) as ppool,
            tc.tile_pool(name="tmp", bufs=1) as tpool,
            tc.tile_pool(name="misc", bufs=1) as mpool,
        ):
            V = nc.vector
            G = nc.gpsimd
            S = nc.scalar

            ps_t = mpool.tile([128, 32], f32)
            is_t = mpool.tile([128, 32], f32)

            pt = {}
            for name, w in IN_SPECS:
                dt_ = f32 if name in F32_NAMES else f16
                pt[name] = ppool.tile([128, w * C], dt_, tag=name, name=name)

            # A: warm the Sin table on ACT while DMAs fill
            warm = mpool.tile([128, 1], f16)
            V.memset(warm[:], 0.0)
            S.activation(warm[:], warm[:], AF.Sin)

            nc.sync.dma_start(ps_t[:], pose_small[:])
            nc.sync.dma_start(is_t[:], init_small[:])
            nc.sync.dma_start(pt["PP"][:], ins["PP"][:])
            nc.sync.dma_start(pt["PR"][:], ins["PR"][:])
            nc.sync.dma_start(pt["RC1"][:], ins["RC1"][:])
            nc.sync.dma_start(pt["RC2"][:], ins["RC2"][:])
            for name, w in IN_SPECS:
                if name in ("PP", "PR", "RC1", "RC2"):
                    continue
                nc.sync.dma_start(pt[name][:], ins[name][:])

            def wtile(w, tag):
                return tpool.tile([128, w * C], f16, tag=tag, name=tag)

            def vtt(out, a, b, op):
                V.tensor_tensor(out=out, in0=a, in1=b, op=op)

            def bc3(ap):
                return ap.rearrange("p (k c) -> p k c", k=1).to_broadcast(
                    (128, 3, C))

            def bc2(ap):
                return ap.rearrange("p (k c) -> p k c", k=1).to_broadcast(
                    (128, 2, C))

            def v3(ap):
                return ap.rearrange("p (k c) -> p k c", c=C)

            # ========== pose path: component-batched tiny fp32, V front ===
            pose_out = mpool.tile([128, 24], f32)

            def P8(tile_):
                return tile_[:].rearrange("p (s c) -> p s c", c=8)

            def PD(tag, w=6):
                return tpool.tile([128, 4 * w], f32, tag="ps_" + tag,
                                  name="ps_" + tag).rearrange(
                    "p (s c) -> p s c", c=w)

            pv = P8(ps_t)
            iv = P8(is_t)
            p_t3 = pv[:, :, 0:3]
            i_t3 = iv[:, :, 0:3]
            p_q3 = pv[:, :, 3:6]
            i_q3 = iv[:, :, 3:6]
            p_w = pv[:, :, 6:7].to_broadcast((128, 4, 3))
            i_w = iv[:, :, 6:7].to_broadcast((128, 4, 3))
            p_ws = pv[:, :, 6]
            i_ws = iv[:, :, 6]

            ddup = PD("ddup")
            idup = PD("idup")
            qdup = PD("qdup")
            udup = PD("udup")
            tdup = PD("tdup")
            wdup = PD("wdup")
            mm1 = PD("mm1", 3)
            mm2 = PD("mm2", 3)
            c2t = PD("c2t", 3)
            qmv = PD("qmv", 3)
            dt_ = PD("dt3", 3)

            def dup(dst, src_ap):
                V.tensor_copy(out=dst[:, :, 0:3], in_=src_ap)
                V.tensor_copy(out=dst[:, :, 3:6], in_=src_ap)

            def crossb(out3, a6, b6):
                # out = a x b via shifted views of duplicated operands
                V.tensor_tensor(out=mm1[:, :, 0:3], in0=a6[:, :, 1:4],
                                in1=b6[:, :, 2:5], op=OP.mult)
                V.tensor_tensor(out=mm2[:, :, 0:3], in0=a6[:, :, 2:5],
                                in1=b6[:, :, 1:4], op=OP.mult)
                V.tensor_tensor(out=out3, in0=mm1[:, :, 0:3],
                                in1=mm2[:, :, 0:3], op=OP.subtract)

            dup(idup, i_q3)
            dup(qdup, p_q3)
            # dt = pose.t - init.t (into ddup twice via one sub + copy)
            V.tensor_tensor(out=dt_[:, :, 0:3], in0=p_t3, in1=i_t3,
                            op=OP.subtract)
            dup(ddup, dt_[:, :, 0:3])
            # u = dt x iv   (= (-iv) x dt = qv x dt)
            crossb(udup[:, :, 0:3], ddup, idup)
            V.tensor_copy(out=udup[:, :, 3:6], in_=udup[:, :, 0:3])
            # c2 = u x iv
            crossb(c2t[:, :, 0:3], udup, idup)
            # tt = dt + 2w*u + 2*c2
            V.tensor_tensor(out=mm1[:, :, 0:3], in0=i_w,
                            in1=udup[:, :, 0:3], op=OP.mult)
            V.tensor_tensor(out=mm1[:, :, 0:3], in0=mm1[:, :, 0:3],
                            in1=mm1[:, :, 0:3], op=OP.add)
            V.tensor_tensor(out=mm2[:, :, 0:3], in0=dt_[:, :, 0:3],
                            in1=mm1[:, :, 0:3], op=OP.add)
            V.scalar_tensor_tensor(out=tdup[:, :, 0:3], in0=c2t[:, :, 0:3],
                                   scalar=2.0, in1=mm2[:, :, 0:3],
                                   op0=OP.mult, op1=OP.add)
            V.tensor_copy(out=tdup[:, :, 3:6], in_=tdup[:, :, 0:3])
            # qm vector part: iw*q2v - q2w*iv - iv x q2v
            crossb(qmv[:, :, 0:3], idup, qdup)
            V.tensor_tensor(out=mm1[:, :, 0:3], in0=i_w, in1=p_q3,
                            op=OP.mult)
            V.tensor_tensor(out=mm2[:, :, 0:3], in0=p_w, in1=i_q3,
                            op=OP.mult)
            V.tensor_tensor(out=mm1[:, :, 0:3], in0=mm1[:, :, 0:3],
                            in1=mm2[:, :, 0:3], op=OP.subtract)
            V.tensor_tensor(out=qmv[:, :, 0:3], in0=mm1[:, :, 0:3],
                            in1=qmv[:, :, 0:3], op=OP.subtract)
            # qmw = iw*pw + dot(iv, q2v)
            V.tensor_tensor(out=mm1[:, :, 0:3], in0=i_q3, in1=p_q3,
                            op=OP.mult)
            qmw = tpool.tile([128, 4], f32, tag="ps_qmw", name="ps_qmw")
            nn_ = tpool.tile([128, 4], f32, tag="ps_nn", name="ps_nn")
            sc1 = tpool.tile([128, 4], f32, tag="ps_sc1", name="ps_sc1")
            sc2 = tpool.tile([128, 4], f32, tag="ps_sc2", name="ps_sc2")
            V.tensor_tensor(out=qmw[:], in0=mm1[:, :, 0], in1=mm1[:, :, 1],
                            op=OP.add)
            V.tensor_tensor(out=qmw[:], in0=qmw[:], in1=mm1[:, :, 2],
                            op=OP.add)
            V.tensor_tensor(out=sc1[:], in0=i_ws, in1=p_ws, op=OP.mult)
            V.tensor_tensor(out=qmw[:], in0=qmw[:], in1=sc1[:], op=OP.add)
            # nn = |qmv|^2
            V.tensor_tensor(out=mm2[:, :, 0:3], in0=qmv[:, :, 0:3],
                            in1=qmv[:, :, 0:3], op=OP.mult)
            V.tensor_tensor(out=nn_[:], in0=mm2[:, :, 0], in1=mm2[:, :, 1],
                            op=OP.add)
            V.tensor_tensor(out=nn_[:], in0=nn_[:], in1=mm2[:, :, 2],
                            op=OP.add)
            # fac = 2*(2-w)*(1 - nn*(2-w)^2/3)
            V.tensor_scalar(out=sc1[:], in0=qmw[:], scalar1=-1.0,
                            scalar2=2.0, op0=OP.mult, op1=OP.add)
            V.tensor_tensor(out=sc2[:], in0=sc1[:], in1=sc1[:], op=OP.mult)
            V.tensor_tensor(out=sc2[:], in0=nn_[:], in1=sc2[:], op=OP.mult)
            V.tensor_scalar(out=sc2[:], in0=sc2[:], scalar1=-1.0 / 3.0,
                            scalar2=1.0, op0=OP.mult, op1=OP.add)
            V.tensor_tensor(out=sc1[:], in0=sc1[:], in1=sc2[:], op=OP.mult)
            V.tensor_tensor(out=sc1[:], in0=sc1[:], in1=sc1[:], op=OP.add)
            # wl = fac * qmv
            facb = sc1[:].rearrange("p (s c) -> p s c", c=1).to_broadcast(
                (128, 4, 3))
            V.tensor_tensor(out=wdup[:, :, 0:3], in0=facb,
                            in1=qmv[:, :, 0:3], op=OP.mult)
            V.tensor_copy(out=wdup[:, :, 3:6], in_=wdup[:, :, 0:3])
            # wxt = wl x tt ; tau = tt - 0.5*wxt ; out
            crossb(mm2[:, :, 0:3], wdup, tdup)
            pout = pose_out[:].rearrange("p (s c) -> p s c", c=6)
            V.scalar_tensor_tensor(out=pout[:, :, 0:3], in0=mm2[:, :, 0:3],
                                   scalar=-0.5, in1=tdup[:, :, 0:3],
                                   op0=OP.mult, op1=OP.add)
            V.tensor_copy(out=pout[:, :, 3:6], in_=wdup[:, :, 0:3])
            nc.scalar.dma_start(res_pose_o[:], pose_out[:])

            # ================= main edge pipeline (packed fp16) ============
            # A: [cth|cph] = Sin(PP + pi/2); [sth|sph] = Sin(PP)
            cc = wtile(2, "cc")
            ss_ = wtile(2, "ssin")
            cth, cph = cc[:, 0:C], cc[:, C:2 * C]
            sth, sph = ss_[:, 0:C], ss_[:, C:2 * C]
            pth_in, pph_in = pt["PP"][:, 0:C], pt["PP"][:, C:2 * C]
            S.activation(cph, pph_in, AF.Sin, bias=HALF_PI)
            S.activation(cth, pth_in, AF.Sin, bias=HALF_PI)
            S.activation(sph, pph_in, AF.Sin)
            S.activation(sth, pth_in, AF.Sin)
            pr = pt["PR"][:]
            vbig = wtile(3, "vbig")
            rc = wtile(1, "rc")
            vtt(rc[:], pr, cph, OP.mult)
            vtt(vbig[:, 0:C], rc[:], cth, OP.mult)            # vx
            vtt(vbig[:, 2 * C:3 * C], pr, sph, OP.mult)       # vz
            vtt(vbig[:, C:2 * C], rc[:], sth, OP.mult)        # vy

            # B: g = R1 v + t1 via column broadcasts
            p1, p2, p3 = wtile(3, "p1"), wtile(3, "p2"), wtile(3, "p3")
            V.tensor_tensor(out=v3(p1[:]), in0=v3(pt["RC1"][:]),
                            in1=bc3(vbig[:, 0:C]), op=OP.mult)
            V.tensor_tensor(out=v3(p2[:]), in0=v3(pt["RC2"][:]),
                            in1=bc3(vbig[:, C:2 * C]), op=OP.mult)
            V.tensor_tensor(out=v3(p3[:]), in0=v3(pt["RC3"][:]),
                            in1=bc3(vbig[:, 2 * C:3 * C]), op=OP.mult)
            vtt(p3[:], p3[:], pt["T1"][:], OP.add)
            vtt(p1[:], p1[:], p2[:], OP.add)
            gbig = p2                                          # reuse
            vtt(gbig[:], p1[:], p3[:], OP.add)

            # C: h = g - t2 ; lx/ly = S rows . h ; hsq for r
            hbig = p1                                          # reuse
            vtt(hbig[:], gbig[:], pt["T2"][:], OP.subtract)
            hx, hy, hz = (hbig[:, 0:C], hbig[:, C:2 * C],
                          hbig[:, 2 * C:3 * C])
            u1, u2 = wtile(2, "u1"), wtile(2, "u2")
            u3 = wtile(2, "u3")
            V.tensor_tensor(out=v3(u1[:]), in0=v3(pt["SC1"][:]),
                            in1=bc2(hx), op=OP.mult)
            V.tensor_tensor(out=v3(u2[:]), in0=v3(pt["SC2"][:]),
                            in1=bc2(hy), op=OP.mult)
            V.tensor_tensor(out=v3(u3[:]), in0=v3(pt["SC3"][:]),
                            in1=bc2(hz), op=OP.mult)
            vtt(u1[:], u1[:], u2[:], OP.add)
            lbig = u2                                          # reuse
            vtt(lbig[:], u1[:], u3[:], OP.add)
            lx, ly = lbig[:, 0:C], lbig[:, C:2 * C]

            # r chain: hsq = h*h ; ssb = [xl+yl | s1+zh]
            hsq = p3                                           # reuse
            vtt(hsq[:, 0:C], hx, hx, OP.mult)
            vtt(hsq[:, C:2 * C], hy, hy, OP.mult)
            vtt(hsq[:, 2 * C:3 * C], hz, hz, OP.mult)
            lsq = u3                                           # reuse
            vtt(lsq[:, 0:C], lx, lx, OP.mult)
            vtt(lsq[:, C:2 * C], ly, ly, OP.mult)
            ssb = wtile(2, "ssb")
            vtt(ssb[:, 0:C], lsq[:, 0:C], lsq[:, C:2 * C], OP.add)
            s1t = wtile(1, "s1t")
            vtt(s1t[:], hsq[:, 0:C], hsq[:, C:2 * C], OP.add)
            vtt(ssb[:, C:2 * C], s1t[:], hsq[:, 2 * C:3 * C], OP.add)
            # [rxy | ro] = Sqrt(ssb), two halves, AF-grouped pipelining
            HW2 = C // 2
            ssbv = ssb[:].rearrange("p (k c) -> p k c", c=C)
            rob = wtile(2, "rob")
            robv = rob[:].rearrange("p (k c) -> p k c", c=C)
            for h in range(2):
                sl = slice(h * HW2, (h + 1) * HW2)
                S.activation(robv[:, :, sl], ssbv[:, :, sl], AF.Sqrt)
            dh = []
            for h in range(2):
                sl = slice(h * HW2, (h + 1) * HW2)
                den = tpool.tile([128, HW2], f16, tag=f"den{h}",
                                 name=f"den{h}")
                vtt(den[:], robv[:, 0, sl], lbig[:, 0:C][:, sl], OP.add)
                V.tensor_scalar(out=den[:], in0=den[:], scalar1=DEN_CLAMP,
                                scalar2=None, op0=OP.max)
                dsq = tpool.tile([128, HW2], f16, tag=f"dsq{h}",
                                 name=f"dsq{h}")
                vtt(dsq[:], den[:], den[:], OP.mult)
                dh.append({"dsq": dsq, "sl": sl})
            # res_r (early, drains while theta chain finishes)
            resb = wtile(2, "resb")
            vtt(resb[:, 0:C], robv[:, 1, :], pt["TC"][:, 0:C], OP.subtract)
            nc.sync.dma_start(res_o[:, 0:C], resb[:, 0:C])
            for h in range(2):
                d = dh[h]
                d["inv"] = tpool.tile([128, HW2], f16, tag=f"inv{h}",
                                      name=f"inv{h}")
                S.activation(d["inv"][:], d["dsq"][:],
                             AF.Abs_reciprocal_sqrt)
            for h in range(2):
                d = dh[h]
                d["yi"] = tpool.tile([128, HW2], f16, tag=f"yi{h}",
                                     name=f"yi{h}")
                vtt(d["yi"][:], lbig[:, C:2 * C][:, d["sl"]], d["inv"][:],
                    OP.mult)
            for h in range(2):
                d = dh[h]
                d["at"] = tpool.tile([128, HW2], f16, tag=f"at{h}",
                                     name=f"at{h}")
                S.activation(d["at"][:], d["yi"][:], AF.Arctan)
            for h in range(2):
                d = dh[h]
                V.scalar_tensor_tensor(
                    out=resb[:, C:2 * C][:, d["sl"]], in0=d["at"][:],
                    scalar=2.0, in1=pt["TC"][:, C:2 * C][:, d["sl"]],
                    op0=OP.mult, op1=OP.subtract)
                nc.sync.dma_start(res_o[:, C:2 * C][:, d["sl"]],
                                  resb[:, C:2 * C][:, d["sl"]])




            # elev residual on GpSimd (f32)
            res_e_t = mpool.tile([128, C], f32)
            G.tensor_tensor(out=res_e_t[:], in0=pt["EA"][:], in1=pt["EI"][:],
                            op=OP.subtract)
            nc.scalar.dma_start(res_e_o[:], res_e_t[:])

    nc.compile()
    return nc


def _get_program():
    if "main" not in _PROGRAM_CACHE:
        _PROGRAM_CACHE["main"] = _build_program()
    return _PROGRAM_CACHE["main"]


def _rot_mats(q):
    x, y, z, w = q[:, 0], q[:, 1], q[:, 2], q[:, 3]
    R = np.empty((q.shape[0], 3, 3), np.float32)
    R[:, 0, 0] = 1 - 2 * (y * y + z * z)
    R[:, 0, 1] = 2 * (x * y - w * z)
    R[:, 0, 2] = 2 * (x * z + w * y)
    R[:, 1, 0] = 2 * (x * y + w * z)
    R[:, 1, 1] = 1 - 2 * (x * x + z * z)
    R[:, 1, 2] = 2 * (y * z - w * x)
    R[:, 2, 0] = 2 * (x * z - w * y)
    R[:, 2, 1] = 2 * (y * z + w * x)
    R[:, 2, 2] = 1 - 2 * (x * x + y * y)
    return R


def kernel(poses, patch_coords, elevation_angle, init_poses,
           init_elevation_angle, target_coords, source_poses_idx,
           target_poses_idx, patch_idx):
    poses = np.asarray(poses, dtype=np.float32)
    patch_coords = np.asarray(patch_coords, dtype=np.float32)
    elevation_angle = np.asarray(elevation_angle, dtype=np.float32)
    init_poses = np.asarray(init_poses, dtype=np.float32)
    init_elevation_angle = np.asarray(init_elevation_angle, dtype=np.float32)
    target_coords = np.asarray(target_coords, dtype=np.float32)
    spi = np.asarray(source_poses_idx, dtype=np.int32)
    tpi = np.asarray(target_poses_idx, dtype=np.int32)
    pix = np.asarray(patch_idx, dtype=np.int32)

    nc = _get_program()

    pose_t = poses[0]
    R1_tbl = _rot_mats(pose_t[:, 3:7])
    R2T_tbl = np.ascontiguousarray(R1_tbl.transpose(0, 2, 1))

    R1 = R1_tbl[spi]
    t1 = pose_t[spi, :3]
    R2T = R2T_tbl[tpi]
    t2 = pose_t[tpi, :3]
    pc = patch_coords[0][pix]
    pph = elevation_angle[0][pix, 0]

    def s16(x):
        return np.asarray(x, dtype=np.float16).reshape(NCORES, 128, C)

    def s32(x):
        return np.asarray(x, dtype=np.float32).reshape(NCORES, 128, C)

    def pack(*planes):
        return np.ascontiguousarray(np.concatenate(planes, axis=2))

    tensors = {
        "PP": pack(s16(pc[:, 1]), s16(pph)),
        "PR": np.ascontiguousarray(s16(pc[:, 0])),
        "RC1": pack(s16(R1[:, 0, 0]), s16(R1[:, 1, 0]), s16(R1[:, 2, 0])),
        "RC2": pack(s16(R1[:, 0, 1]), s16(R1[:, 1, 1]), s16(R1[:, 2, 1])),
        "RC3": pack(s16(R1[:, 0, 2]), s16(R1[:, 1, 2]), s16(R1[:, 2, 2])),
        "T1": pack(s16(t1[:, 0]), s16(t1[:, 1]), s16(t1[:, 2])),
        "T2": pack(s16(t2[:, 0]), s16(t2[:, 1]), s16(t2[:, 2])),
        "SC1": pack(s16(R2T[:, 0, 0]), s16(R2T[:, 1, 0])),
        "SC2": pack(s16(R2T[:, 0, 1]), s16(R2T[:, 1, 1])),
        "SC3": pack(s16(R2T[:, 0, 2]), s16(R2T[:, 1, 2])),
        "EA": np.ascontiguousarray(s32(elevation_angle[0][:, 0])),
        "EI": np.ascontiguousarray(s32(init_elevation_angle[0][:, 0])),
        "TC": pack(s16(target_coords[0][:, 0]), s16(target_coords[0][:, 1])),
    }

    in_maps = []
    for c in range(NCORES):
        ps = np.zeros((512, 8), np.float32)
        ps[:, :7] = poses[0, c * 512:(c + 1) * 512]
        ini = np.zeros((512, 8), np.float32)
        ini[:, :7] = init_poses[0, c * 512:(c + 1) * 512]
        im = {name: tensors[name][c] for name, _ in IN_SPECS}
        im["pose_small"] = ps.reshape(128, 32)
        im["init_small"] = ini.reshape(128, 32)
        in_maps.append(im)

    res = run_bass_kernel_spmd(nc, in_maps, list(range(NCORES)))

    rb = np.stack([res.results[c]["res_o"] for c in range(NCORES)])
    rr = rb[:, :, 0:C].reshape(E).astype(np.float32)
    rt = rb[:, :, C:2 * C].reshape(E).astype(np.float32)
    re_ = np.stack([res.results[c]["res_e_o"] for c in range(NCORES)])
    res_proj = np.stack([rr, rt], axis=-1)
    res_elev = re_.reshape(E).astype(np.float32)
    res_pose = np.zeros((P, 6), np.float32)
    for c in range(NCORES):
        res_pose[c * 512:(c + 1) * 512] = \
            res.results[c]["res_pose_o"].reshape(128, 4, 6).reshape(512, 6)

    return np.concatenate([res_proj.reshape(-1), res_pose.reshape(-1),
                           res_elev]).reshape(1, -1).astype(np.float32)
